# revision 31
# baseline (speedup 1.0000x reference)
"""Trainium2 Bass kernel for the 4-channel bleed-correction model
(nn_Neural_44770739094212, gnn_message_passing).

Math (per batch image, channels C=4, 3x3 kernels, SAME padding):
  for each channel i, neighbors j = i+-1:
      bleed_i += conv(s_j, K[kc]) + conv((s_j^0.5 * s_i)^(2/3), K[ki])
  out_i = s_i - bleed_i

Strategy:
  - Pure data parallel over batch: B=32 -> 4 images per core x 8 cores.
  - 3x3 conv = 3 banded-matrix matmuls on the tensor engine (fp32r):
    contraction over H rows via a 128x128 banded lhsT holding the kernel's
    column taps; the W-direction taps are handled by shifting the rhs /
    psum column windows.  All terms of one output channel accumulate into
    a single PSUM bank.
  - Interaction term (s_j^0.5 * s_i)^(2/3) = s_j^(1/3) * s_i^(2/3):
    a_c = exp(ln(s_c)/3) on the scalar engine, b_c = a_c^2 on gpsimd,
    e_ij = a_j * b_i on the vector engine.
  - out_i = s_i - bleed_i fused into one vector op reading PSUM.
"""

import sys

for _p in ("/opt/trn_rl_repo",):
    if _p not in sys.path:
        sys.path.insert(0, _p)

import numpy as np

from concourse import bass, tile, mybir
from concourse.bass_utils import run_bass_kernel_spmd

f32 = mybir.dt.float32
bf16 = mybir.dt.bfloat16
ACT = mybir.ActivationFunctionType
ALU = mybir.AluOpType

C = 4
N_CORES = 8
# (i, j, k_contrib, k_inter) in reference kidx order
LN_BIAS = 1e-30
PAIRS = [(0, 1, 0, 1), (1, 0, 2, 3), (1, 2, 4, 5), (2, 1, 6, 7), (2, 3, 8, 9), (3, 2, 10, 11)]


def _chunks(H):
    """Output-row chunks: (out_start, n_out, in_start, n_in, variant).
    variant 0 = top (in window starts at out row), 1 = mid (starts 1 above)."""
    ch = [(0, min(127, H), 0, min(128, H), 0)]
    o = ch[0][1]
    while o < H:
        n_out = min(126, H - o)
        i0 = o - 1
        n_in = min(n_out + 2, H - i0)
        ch.append((o, n_out, i0, n_in, 1))
        o += n_out
    return ch


def _band_mats(kernels):
    """bands[v, t, dw] in R^{128x128}: lhsT[ti, to] = -K_t[ti - to + off_v, dw]
    where off=1 for the top variant (v=0) and off=0 for mid (v=1).
    Negated so that PSUM accumulates s_i - bleed directly (see _ident_mats)."""
    bands = np.zeros((2, 12, 3, 128, 128), np.float32)
    for v, off in ((0, 1), (1, 0)):
        for t in range(12):
            for dw in range(3):
                m = np.zeros((128, 128), np.float32)
                for dh in range(3):
                    m -= kernels[t, dh, dw] * np.eye(128, dtype=np.float32, k=off - dh)
                bands[v, t, dw] = m
    return bands


def _ident_mats():
    """Shifted identity lhsT adding s_i into the PSUM group: out row `to`
    reads in-window row `to + ro` where ro = 0 (top) / 1 (mid)."""
    idm = np.zeros((2, 128, 128), np.float32)
    idm[0] = np.eye(128, dtype=np.float32, k=0)
    idm[1] = np.eye(128, dtype=np.float32, k=-1)
    return idm


def _pack_bands(kernels):
    """Pack the 72 band matrices + 2 identities column-wise into single
    [128, 74*128] host arrays so they load with one DMA.  Slot order:
    idx = (v*12 + t)*3 + dw for bands, then 72+v for identities."""
    bands = _band_mats(kernels).reshape(72, 128, 128)
    idm = _ident_mats()
    allm = np.concatenate([bands, idm], axis=0)  # [74, 128, 128]
    return np.ascontiguousarray(allm.transpose(1, 0, 2).reshape(128, 74 * 128))


def _split_multi_waits(nc, limit=1):
    """This walrus build accepts at most one sync wait per instruction
    (CTRL templates); move excess waits onto preceding same-engine NoOps."""
    for fn in nc.m.functions:
        for bb in fn.blocks:
            new_list = []
            changed = False
            for inst in bb.instructions:
                si = inst.sync_info
                if si is not None and si.on_wait is not None and len(si.on_wait) > limit:
                    waits = list(si.on_wait)
                    keep, excess = waits[-limit:], waits[:-limit]
                    for i, w in enumerate(excess):
                        nop = mybir.InstNoOp(name=f"{inst.name}-wsplit{i}")
                        nop.engine = inst.engine
                        nop.sync_info = mybir.SyncInfo(on_wait=[w], on_update=[])
                        new_list.append(nop)
                    inst.sync_info = mybir.SyncInfo(
                        on_wait=keep, on_update=list(si.on_update or [])
                    )
                    changed = True
                new_list.append(inst)
            if changed:
                bb.instructions = new_list


def build_nc(B_loc, H, W, split_waits=True):
    nc = bass.Bass(trn_type="TRN2", debug=False, target_bir_lowering=False)
    # register a tiny Ln bias so ln(0) can't produce -inf/nan
    _bias_t = nc.alloc_sbuf_tensor("const-ln-bias", [128, 1], f32)
    nc.gpsimd.memset(_bias_t.ap(), LN_BIAS)
    nc.const_aps.aps[(f32, LN_BIAS)] = _bias_t.ap()
    nc.all_engine_barrier()
    src = nc.dram_tensor("src", [B_loc, H, C, W], bf16, kind="ExternalInput")
    band = nc.dram_tensor("band", [128, 74 * 128], bf16, kind="ExternalInput")
    out = nc.dram_tensor("out", [B_loc, H, C, W], f32, kind="ExternalOutput")
    chunks = _chunks(H)

    with tile.TileContext(nc) as tc:
        with (
            tc.tile_pool(name="bands", bufs=1) as bpool,
            tc.tile_pool(name="data", bufs=2) as dpool,
            tc.tile_pool(name="psum", bufs=2, space="PSUM") as ppool,
        ):
            # all 72 band matrices + 2 identities in one tile, 4 split DMAs
            bandmeg = bpool.tile([128, 74 * 128], bf16, tag="bandmeg", bufs=1)
            nc.sync.dma_start(out=bandmeg[:, :], in_=band[:, :])

            def lhs_slice(v, t, dw):
                idx = 72 + v if t is None else (v * 12 + t) * 3 + dw
                return bandmeg[:, idx * 128 : (idx + 1) * 128]

            pending_stores = []
            pending_copies = []

            def flush_copies():
                for ps_, poff_, omeg_, i_, n_out_ in pending_copies:
                    nc.vector.tensor_copy(
                        omeg_[0:n_out_, i_ * W : (i_ + 1) * W],
                        ps_[poff_ : poff_ + n_out_, 0:W],
                    )
                pending_copies.clear()

            def flush_store():
                omeg_, b_, o0_, n_out_ = pending_stores.pop(0)
                # split across DMA engines: each piece lands on its own engine
                step = n_out_
                for p0 in range(0, n_out_, step):
                    rows = min(step, n_out_ - p0)
                    nc.scalar.dma_start(
                        out=out[b_, o0_ + p0 : o0_ + p0 + rows, :, :].rearrange(
                            "h c w -> h (c w)"
                        ),
                        in_=omeg_[p0 : p0 + rows, :],
                    )

            for b in range(B_loc):
                for (o0, n_out, i0, n_in, var) in chunks:
                    # one DMA for all 4 channels: smeg[h, c*W + w] = src[c,b,i0+h,w]
                    smeg = dpool.tile([128, C * W], bf16, tag="smeg", bufs=6)
                    lstep = n_in
                    for p0 in range(0, n_in, lstep):
                        rows = min(lstep, n_in - p0)
                        nc.sync.dma_start(
                            out=smeg[p0 : p0 + rows, :],
                            in_=src[b, i0 + p0 : i0 + p0 + rows, :, :].rearrange(
                                "h c w -> h (c w)"
                            ),
                        )
                    # stores ride the SP queue 3 chunks behind the loads, so
                    # their data is long since ready and they never block
                    if len(pending_stores) > 3:
                        flush_store()
                    # interaction term e_ij = s_j^(1/3) s_i^(2/3)
                    #                       = exp((L_j + 2 L_i)/3),  L = ln(s).
                    # Packed by neighbor dir: emegA col i*W = e_{i,i+1},
                    # emegB col j*W = e_{j+1,j}.
                    lmeg = dpool.tile([128, C * W], f32, tag="lmeg", bufs=4)
                    nc.scalar.activation(lmeg[0:n_in, :], smeg[0:n_in, :], ACT.Ln, bias=LN_BIAS)
                    E = (C - 1) * W
                    uA = dpool.tile([128, E], f32, tag="uA", bufs=3)
                    nc.vector.scalar_tensor_tensor(
                        uA[0:n_in, :], lmeg[0:n_in, 0:E], 2.0, lmeg[0:n_in, W : C * W],
                        op0=ALU.mult, op1=ALU.add,
                    )
                    uB = dpool.tile([128, E], f32, tag="uB", bufs=3)
                    nc.vector.scalar_tensor_tensor(
                        uB[0:n_in, :], lmeg[0:n_in, W : C * W], 2.0, lmeg[0:n_in, 0:E],
                        op0=ALU.mult, op1=ALU.add,
                    )
                    emegA = dpool.tile([128, E], bf16, tag="emegA", bufs=6)
                    nc.scalar.activation(emegA[0:n_in, :], uA[0:n_in, :], ACT.Exp, scale=1.0 / 3.0)
                    emegB = dpool.tile([128, E], bf16, tag="emegB", bufs=6)
                    nc.scalar.activation(emegB[0:n_in, :], uB[0:n_in, :], ACT.Exp, scale=1.0 / 3.0)
                    # previous chunk's psum drains go AFTER this chunk's u ops on
                    # the vector queue, so prep is never serialized behind the PE
                    flush_copies()

                    omeg = dpool.tile([128, C * W], f32, tag="omeg", bufs=5)
                    # small tail chunk: pack the 4 channels into one PSUM bank
                    # at 32-aligned column groups so the PE runs them 4-wide
                    tail = n_out <= 32
                    ps_tail = None
                    if tail:
                        ps_tail = ppool.tile([128, W], f32, tag="ps0", name="ps_tail")
                    for i in range(C):
                        terms = []
                        for (ii, j, kc, ki) in PAIRS:
                            if ii == i:
                                if j == i + 1:
                                    e_col = i * W
                                    e_src = emegA
                                else:
                                    e_col = j * W
                                    e_src = emegB
                                terms += [(smeg, j * W, kc), (e_src, e_col, ki)]
                        if tail:
                            ps, p_off, m_out = ps_tail, 32 * i, n_out
                        else:
                            ps = ppool.tile([128, W], f32, tag=f"ps{i}", name=f"ps{i}")
                            p_off, m_out = 0, 128
                        # identity matmul first: psum = shifted_I @ s_i; it has
                        # start=True and full width so it initializes every psum
                        # element the later partial-width band matmuls touch.
                        # Band matrices are negated, so psum ends as s_i - bleed.
                        mms = [(smeg, i * W, None, 1)] + [
                            (xt, col, t, dw) for (xt, col, t) in terms for dw in (1, 0, 2)
                        ]
                        for idx, (xt, col, t, dw) in enumerate(mms):
                            if dw == 1:
                                oc, ic, fl = 0, 0, W
                            elif dw == 0:
                                oc, ic, fl = 1, 0, W - 1
                            else:
                                oc, ic, fl = 0, 1, W - 1
                            nc.tensor.matmul(
                                ps[p_off : p_off + m_out, oc : oc + fl],
                                lhsT=lhs_slice(var, t, dw)[0:n_in, 0:m_out],
                                rhs=xt[0:n_in, col + ic : col + ic + fl],
                                start=(idx == 0),
                                stop=(idx == len(mms) - 1),
                                tile_position=(0, p_off) if tail else None,
                            )
                        pending_copies.append((ps, p_off, omeg, i, n_out))
                    pending_stores.append(
                        (omeg, b, o0, n_out)
                    )

            flush_copies()
            while pending_stores:
                flush_store()

    if split_waits:
        _split_multi_waits(nc)
    return nc


def _install_axon_profile_hook():
    """Provide antenv.axon_hooks (absent in this image) so
    run_bass_kernel_spmd(trace=True) can capture NTFF profiles via the
    axon sidechannel.  Only used by test.py; grading never passes trace."""
    import types
    import ctypes
    import contextlib

    if "antenv.axon_hooks" in sys.modules:
        return
    try:
        lib = ctypes.CDLL("/opt/axon/libaxon_pjrt.so")
    except OSError:
        return
    if not hasattr(lib, "axon_start_nrt_profile"):
        return
    lib.axon_start_nrt_profile.argtypes = [ctypes.POINTER(ctypes.c_int64), ctypes.c_size_t]
    lib.axon_start_nrt_profile.restype = ctypes.c_int64
    lib.axon_stop_nrt_profile.argtypes = [ctypes.c_char_p]
    lib.axon_stop_nrt_profile.restype = ctypes.c_int64

    @contextlib.contextmanager
    def _hook(output_dir, device_ids):
        import jax

        jax.devices()
        if device_ids:
            ids = (ctypes.c_int64 * len(device_ids))(*device_ids)
            rc = lib.axon_start_nrt_profile(ids, len(device_ids))
        else:
            rc = lib.axon_start_nrt_profile(None, 0)
        if rc != 0:
            raise RuntimeError(f"axon_start_nrt_profile rc={rc}")
        try:
            yield
        finally:
            n = lib.axon_stop_nrt_profile(str(output_dir).encode())
            print(f"profile: {n} file(s) written to {output_dir}")

    mod = types.ModuleType("antenv.axon_hooks")
    mod.get_axon_ntff_profile_hook = lambda: _hook
    mod.set_axon_ntff_profile_hook = lambda h: None
    sys.modules["antenv.axon_hooks"] = mod


_NC_CACHE = {}


def kernel(sources, kernels, trace=False):
    sources = np.asarray(sources)
    kernels = np.asarray(kernels, dtype=np.float32)
    _c, B, H, W, _one = sources.shape
    B_loc = B // N_CORES
    key = (B_loc, H, W)
    if key not in _NC_CACHE:
        _NC_CACHE[key] = build_nc(B_loc, H, W)
    nc = _NC_CACHE[key]

    np_bf16 = mybir.dt.np(bf16)
    bands = _pack_bands(kernels).astype(np_bf16)
    # [C,B,H,W] -> [B,H,C,W] so per-chunk DMAs are fully contiguous in HBM
    src = sources.astype(np.float32)[..., 0].astype(np_bf16).transpose(1, 2, 0, 3)
    in_maps = [
        {
            "src": np.ascontiguousarray(src[m * B_loc : (m + 1) * B_loc]),
            "band": bands,
        }
        for m in range(N_CORES)
    ]
    kwargs = {}
    if trace:
        _install_axon_profile_hook()
        import os

        tmpdir = "/root/problem/trace_out"
        os.makedirs(tmpdir, exist_ok=True)
        kwargs["tmpdir"] = tmpdir
    res = run_bass_kernel_spmd(nc, in_maps, core_ids=list(range(N_CORES)), trace=trace, **kwargs)
    # per-core [B_loc,H,C,W] -> gather on B -> [C,B,H,W,1]
    out = np.concatenate([np.asarray(r["out"]) for r in res.results], axis=0)
    out = out.transpose(2, 0, 1, 3)[..., None].astype(np.float32)
    if trace:
        return out, res
    return out


# revision 32
# speedup vs baseline: 1.1337x; 1.1337x over previous
"""Trainium2 Bass kernel for the 4-channel bleed-correction model
(nn_Neural_44770739094212, gnn_message_passing).

Math (per batch image, channels C=4, 3x3 kernels, SAME padding):
  for each channel i, neighbors j = i+-1:
      bleed_i += conv(s_j, K[kc]) + conv((s_j^0.5 * s_i)^(2/3), K[ki])
  out_i = s_i - bleed_i

Strategy:
  - Pure data parallel over batch: B=32 -> 4 images per core x 8 cores.
  - 3x3 conv = 3 banded-matrix matmuls on the tensor engine (fp32r):
    contraction over H rows via a 128x128 banded lhsT holding the kernel's
    column taps; the W-direction taps are handled by shifting the rhs /
    psum column windows.  All terms of one output channel accumulate into
    a single PSUM bank.
  - Interaction term (s_j^0.5 * s_i)^(2/3) = s_j^(1/3) * s_i^(2/3):
    a_c = exp(ln(s_c)/3) on the scalar engine, b_c = a_c^2 on gpsimd,
    e_ij = a_j * b_i on the vector engine.
  - out_i = s_i - bleed_i fused into one vector op reading PSUM.
"""

import sys

for _p in ("/opt/trn_rl_repo",):
    if _p not in sys.path:
        sys.path.insert(0, _p)

import numpy as np

from concourse import bass, tile, mybir
from concourse.bass_utils import run_bass_kernel_spmd

f32 = mybir.dt.float32
bf16 = mybir.dt.bfloat16
ACT = mybir.ActivationFunctionType
ALU = mybir.AluOpType

C = 4
N_CORES = 8
# (i, j, k_contrib, k_inter) in reference kidx order
LN_BIAS = 1e-30
PAIRS = [(0, 1, 0, 1), (1, 0, 2, 3), (1, 2, 4, 5), (2, 1, 6, 7), (2, 3, 8, 9), (3, 2, 10, 11)]


def _chunks(H):
    """Output-row chunks: (out_start, n_out, in_start, n_in, variant).
    variant 0 = top (in window starts at out row), 1 = mid (starts 1 above)."""
    ch = [(0, min(127, H), 0, min(128, H), 0)]
    o = ch[0][1]
    while o < H:
        n_out = min(126, H - o)
        i0 = o - 1
        n_in = min(n_out + 2, H - i0)
        ch.append((o, n_out, i0, n_in, 1))
        o += n_out
    return ch


def _band_mats(kernels):
    """bands[v, t, dw] in R^{128x128}: lhsT[ti, to] = -K_t[ti - to + off_v, dw]
    where off=1 for the top variant (v=0) and off=0 for mid (v=1).
    Negated so that PSUM accumulates s_i - bleed directly (see _ident_mats)."""
    bands = np.zeros((2, 12, 3, 128, 128), np.float32)
    for v, off in ((0, 1), (1, 0)):
        for t in range(12):
            for dw in range(3):
                m = np.zeros((128, 128), np.float32)
                for dh in range(3):
                    m -= kernels[t, dh, dw] * np.eye(128, dtype=np.float32, k=off - dh)
                bands[v, t, dw] = m
    return bands


def _ident_mats():
    """Shifted identity lhsT adding s_i into the PSUM group: out row `to`
    reads in-window row `to + ro` where ro = 0 (top) / 1 (mid)."""
    idm = np.zeros((2, 128, 128), np.float32)
    idm[0] = np.eye(128, dtype=np.float32, k=0)
    idm[1] = np.eye(128, dtype=np.float32, k=-1)
    return idm


def _pack_bands(kernels):
    """Pack the 72 band matrices + 2 identities column-wise into single
    [128, 74*128] host arrays so they load with one DMA.  Slot order:
    idx = (v*12 + t)*3 + dw for bands, then 72+v for identities."""
    bands = _band_mats(kernels).reshape(72, 128, 128)
    idm = _ident_mats()
    allm = np.concatenate([bands, idm], axis=0)  # [74, 128, 128]
    return np.ascontiguousarray(allm.transpose(1, 0, 2).reshape(128, 74 * 128))


def _split_multi_waits(nc, limit=1):
    """This walrus build accepts at most one sync wait per instruction
    (CTRL templates); move excess waits onto preceding same-engine NoOps."""
    for fn in nc.m.functions:
        for bb in fn.blocks:
            new_list = []
            changed = False
            for inst in bb.instructions:
                si = inst.sync_info
                if si is not None and si.on_wait is not None and len(si.on_wait) > limit:
                    waits = list(si.on_wait)
                    keep, excess = waits[-limit:], waits[:-limit]
                    for i, w in enumerate(excess):
                        nop = mybir.InstNoOp(name=f"{inst.name}-wsplit{i}")
                        nop.engine = inst.engine
                        nop.sync_info = mybir.SyncInfo(on_wait=[w], on_update=[])
                        new_list.append(nop)
                    inst.sync_info = mybir.SyncInfo(
                        on_wait=keep, on_update=list(si.on_update or [])
                    )
                    changed = True
                new_list.append(inst)
            if changed:
                bb.instructions = new_list


def build_nc(B_loc, H, W, split_waits=True):
    nc = bass.Bass(trn_type="TRN2", debug=False, target_bir_lowering=False)
    # register a tiny Ln bias so ln(0) can't produce -inf/nan
    _bias_t = nc.alloc_sbuf_tensor("const-ln-bias", [128, 1], f32)
    nc.gpsimd.memset(_bias_t.ap(), LN_BIAS)
    nc.const_aps.aps[(f32, LN_BIAS)] = _bias_t.ap()
    nc.all_engine_barrier()
    src = nc.dram_tensor("src", [B_loc, H, C, W], bf16, kind="ExternalInput")
    band = nc.dram_tensor("band", [128, 74 * 128], bf16, kind="ExternalInput")
    out = nc.dram_tensor("out", [B_loc, H, C, W], f32, kind="ExternalOutput")
    chunks = _chunks(H)

    with tile.TileContext(nc) as tc:
        with (
            tc.tile_pool(name="bands", bufs=1) as bpool,
            tc.tile_pool(name="data", bufs=2) as dpool,
            tc.tile_pool(name="psum", bufs=2, space="PSUM") as ppool,
        ):
            # all 72 band matrices + 2 identities in one tile, 4 split DMAs
            bandmeg = bpool.tile([128, 74 * 128], bf16, tag="bandmeg", bufs=1)
            nc.sync.dma_start(out=bandmeg[:, :], in_=band[:, :])

            def lhs_slice(v, t, dw):
                idx = 72 + v if t is None else (v * 12 + t) * 3 + dw
                return bandmeg[:, idx * 128 : (idx + 1) * 128]

            pending_stores = []
            pending_copies = []

            def flush_copies():
                for ps_, poff_, omeg_, i_, n_out_ in pending_copies:
                    nc.vector.tensor_copy(
                        omeg_[0:n_out_, i_ * W : (i_ + 1) * W],
                        ps_[poff_ : poff_ + n_out_, 0:W],
                    )
                pending_copies.clear()

            def flush_store():
                omeg_, b_, o0_, n_out_ = pending_stores.pop(0)
                # split across DMA engines: each piece lands on its own engine
                step = (n_out_ + 1) // 2
                for p0 in range(0, n_out_, step):
                    rows = min(step, n_out_ - p0)
                    nc.scalar.dma_start(
                        out=out[b_, o0_ + p0 : o0_ + p0 + rows, :, :].rearrange(
                            "h c w -> h (c w)"
                        ),
                        in_=omeg_[p0 : p0 + rows, :],
                    )

            for b in range(B_loc):
                for (o0, n_out, i0, n_in, var) in chunks:
                    # one DMA for all 4 channels: smeg[h, c*W + w] = src[c,b,i0+h,w]
                    smeg = dpool.tile([128, C * W], bf16, tag="smeg", bufs=6)
                    lstep = n_in
                    for p0 in range(0, n_in, lstep):
                        rows = min(lstep, n_in - p0)
                        nc.sync.dma_start(
                            out=smeg[p0 : p0 + rows, :],
                            in_=src[b, i0 + p0 : i0 + p0 + rows, :, :].rearrange(
                                "h c w -> h (c w)"
                            ),
                        )
                    # stores ride the SP queue 3 chunks behind the loads, so
                    # their data is long since ready and they never block
                    if len(pending_stores) > 3:
                        flush_store()
                    # interaction term e_ij = s_j^(1/3) s_i^(2/3)
                    #                       = exp((L_j + 2 L_i)/3),  L = ln(s).
                    # Packed by neighbor dir: emegA col i*W = e_{i,i+1},
                    # emegB col j*W = e_{j+1,j}.
                    lmeg = dpool.tile([128, C * W], f32, tag="lmeg", bufs=4)
                    nc.scalar.activation(lmeg[0:n_in, :], smeg[0:n_in, :], ACT.Ln, bias=LN_BIAS)
                    E = (C - 1) * W
                    uA = dpool.tile([128, E], f32, tag="uA", bufs=3)
                    nc.vector.scalar_tensor_tensor(
                        uA[0:n_in, :], lmeg[0:n_in, 0:E], 2.0, lmeg[0:n_in, W : C * W],
                        op0=ALU.mult, op1=ALU.add,
                    )
                    uB = dpool.tile([128, E], f32, tag="uB", bufs=3)
                    nc.vector.scalar_tensor_tensor(
                        uB[0:n_in, :], lmeg[0:n_in, W : C * W], 2.0, lmeg[0:n_in, 0:E],
                        op0=ALU.mult, op1=ALU.add,
                    )
                    emegA = dpool.tile([128, E], bf16, tag="emegA", bufs=6)
                    nc.scalar.activation(emegA[0:n_in, :], uA[0:n_in, :], ACT.Exp, scale=1.0 / 3.0)
                    emegB = dpool.tile([128, E], bf16, tag="emegB", bufs=6)
                    nc.scalar.activation(emegB[0:n_in, :], uB[0:n_in, :], ACT.Exp, scale=1.0 / 3.0)
                    # previous chunk's psum drains go AFTER this chunk's u ops on
                    # the vector queue, so prep is never serialized behind the PE
                    flush_copies()

                    omeg = dpool.tile([128, C * W], f32, tag="omeg", bufs=5)
                    # small tail chunk: pack the 4 channels into one PSUM bank
                    # at 32-aligned column groups so the PE runs them 4-wide
                    tail = n_out <= 32
                    ps_tail = None
                    if tail:
                        ps_tail = ppool.tile([128, W], f32, tag="ps0", name="ps_tail")
                    for i in range(C):
                        terms = []
                        for (ii, j, kc, ki) in PAIRS:
                            if ii == i:
                                if j == i + 1:
                                    e_col = i * W
                                    e_src = emegA
                                else:
                                    e_col = j * W
                                    e_src = emegB
                                terms += [(smeg, j * W, kc), (e_src, e_col, ki)]
                        if tail:
                            ps, p_off, m_out = ps_tail, 32 * i, n_out
                        else:
                            ps = ppool.tile([128, W], f32, tag=f"ps{i}", name=f"ps{i}")
                            p_off, m_out = 0, 128
                        # identity matmul first: psum = shifted_I @ s_i; it has
                        # start=True and full width so it initializes every psum
                        # element the later partial-width band matmuls touch.
                        # Band matrices are negated, so psum ends as s_i - bleed.
                        mms = [(smeg, i * W, None, 1)] + [
                            (xt, col, t, dw) for (xt, col, t) in terms for dw in (1, 0, 2)
                        ]
                        for idx, (xt, col, t, dw) in enumerate(mms):
                            if dw == 1:
                                oc, ic, fl = 0, 0, W
                            elif dw == 0:
                                oc, ic, fl = 1, 0, W - 1
                            else:
                                oc, ic, fl = 0, 1, W - 1
                            nc.tensor.matmul(
                                ps[p_off : p_off + m_out, oc : oc + fl],
                                lhsT=lhs_slice(var, t, dw)[0:n_in, 0:m_out],
                                rhs=xt[0:n_in, col + ic : col + ic + fl],
                                start=(idx == 0),
                                stop=(idx == len(mms) - 1),
                                tile_position=(0, p_off) if tail else None,
                            )
                        pending_copies.append((ps, p_off, omeg, i, n_out))
                    pending_stores.append(
                        (omeg, b, o0, n_out)
                    )

            flush_copies()
            while pending_stores:
                flush_store()

    if split_waits:
        _split_multi_waits(nc)
    return nc


def _install_axon_profile_hook():
    """Provide antenv.axon_hooks (absent in this image) so
    run_bass_kernel_spmd(trace=True) can capture NTFF profiles via the
    axon sidechannel.  Only used by test.py; grading never passes trace."""
    import types
    import ctypes
    import contextlib

    if "antenv.axon_hooks" in sys.modules:
        return
    try:
        lib = ctypes.CDLL("/opt/axon/libaxon_pjrt.so")
    except OSError:
        return
    if not hasattr(lib, "axon_start_nrt_profile"):
        return
    lib.axon_start_nrt_profile.argtypes = [ctypes.POINTER(ctypes.c_int64), ctypes.c_size_t]
    lib.axon_start_nrt_profile.restype = ctypes.c_int64
    lib.axon_stop_nrt_profile.argtypes = [ctypes.c_char_p]
    lib.axon_stop_nrt_profile.restype = ctypes.c_int64

    @contextlib.contextmanager
    def _hook(output_dir, device_ids):
        import jax

        jax.devices()
        if device_ids:
            ids = (ctypes.c_int64 * len(device_ids))(*device_ids)
            rc = lib.axon_start_nrt_profile(ids, len(device_ids))
        else:
            rc = lib.axon_start_nrt_profile(None, 0)
        if rc != 0:
            raise RuntimeError(f"axon_start_nrt_profile rc={rc}")
        try:
            yield
        finally:
            n = lib.axon_stop_nrt_profile(str(output_dir).encode())
            print(f"profile: {n} file(s) written to {output_dir}")

    mod = types.ModuleType("antenv.axon_hooks")
    mod.get_axon_ntff_profile_hook = lambda: _hook
    mod.set_axon_ntff_profile_hook = lambda h: None
    sys.modules["antenv.axon_hooks"] = mod


_NC_CACHE = {}


def kernel(sources, kernels, trace=False):
    sources = np.asarray(sources)
    kernels = np.asarray(kernels, dtype=np.float32)
    _c, B, H, W, _one = sources.shape
    B_loc = B // N_CORES
    key = (B_loc, H, W)
    if key not in _NC_CACHE:
        _NC_CACHE[key] = build_nc(B_loc, H, W)
    nc = _NC_CACHE[key]

    np_bf16 = mybir.dt.np(bf16)
    bands = _pack_bands(kernels).astype(np_bf16)
    # [C,B,H,W] -> [B,H,C,W] so per-chunk DMAs are fully contiguous in HBM
    src = sources.astype(np.float32)[..., 0].astype(np_bf16).transpose(1, 2, 0, 3)
    in_maps = [
        {
            "src": np.ascontiguousarray(src[m * B_loc : (m + 1) * B_loc]),
            "band": bands,
        }
        for m in range(N_CORES)
    ]
    kwargs = {}
    if trace:
        _install_axon_profile_hook()
        import os

        tmpdir = "/root/problem/trace_out"
        os.makedirs(tmpdir, exist_ok=True)
        kwargs["tmpdir"] = tmpdir
    res = run_bass_kernel_spmd(nc, in_maps, core_ids=list(range(N_CORES)), trace=trace, **kwargs)
    # per-core [B_loc,H,C,W] -> gather on B -> [C,B,H,W,1]
    out = np.concatenate([np.asarray(r["out"]) for r in res.results], axis=0)
    out = out.transpose(2, 0, 1, 3)[..., None].astype(np.float32)
    if trace:
        return out, res
    return out
